# revision 3
# baseline (speedup 1.0000x reference)
"""Per-expert SwiGLU FFN (MoE) kernel for Trainium2, expert-parallel over 8 cores.

Reference computation (per expert e):
    y1 = x[e] @ W_fc1[e]          # [T,D] @ [D,H] -> [T,H]
    y2 = x[e] @ W_fc2[e]
    y  = silu(y1) * y2
    out[e] = y @ W_fc3[e]         # [T,H] @ [H,D] -> [T,D]

Shapes: E=8 experts, T=1024 tokens, D=2048, H=5632. One expert per core.

Per-core dataflow:
  Phase 0: DMA x naturally (T on partitions), PE-transpose 128x128 blocks to
           build xT (D on partitions) as fp16 in SBUF.
  Phase A: per h-tile (44 of 128): stream W1/W2 column strips, cast fp16,
           mm1/mm2 accumulate over D into PSUM (fp32), silu in-place on PSUM
           (ScalarE), multiply (VectorE) -> resident y strip [H,T] fp16.
  Phase B: per d-block (8 of 256): stream W3 column block, cast fp16,
           mm3 accumulates over all 44 h-tiles into PSUM, evict to DRAM out.

All matmuls run fp16 (1 cycle/row on the PE vs 4 for fp32) with fp32 PSUM
accumulation.
"""

import numpy as np

import concourse.bass as bass
import concourse.mybir as mybir
import concourse.tile as tile
from concourse import bacc
from concourse.bass_utils import run_bass_kernel_spmd
from concourse.masks import make_identity

E, T, D, H = 8, 1024, 2048, 5632
P = 128
DT = D // P   # 16 d-tiles
HT = H // P   # 44 h-tiles
TT = T // P   # 8 t-tiles
DB = 256      # phase-B d-block width
NDB = D // DB  # 8
CH = 4        # w3 h-tiles per staged DMA chunk
NCH = HT // CH  # 11

F32 = mybir.dt.float32
F16 = mybir.dt.float16

_cache = {}


def _build():
    nc = bacc.Bacc("TRN2", target_bir_lowering=False, debug=False)
    x = nc.dram_tensor("x", [T, D], F32, kind="ExternalInput").ap()
    w1 = nc.dram_tensor("w1", [D, H], F32, kind="ExternalInput").ap()
    w2 = nc.dram_tensor("w2", [D, H], F32, kind="ExternalInput").ap()
    w3 = nc.dram_tensor("w3", [H, D], F32, kind="ExternalInput").ap()
    out = nc.dram_tensor("out", [T, D], F32, kind="ExternalOutput").ap()

    with tile.TileContext(nc) as tc:
        with (
            tc.tile_pool(name="y", bufs=1) as ypool,
            tc.tile_pool(name="const", bufs=1) as cpool,
        ):
            ident = cpool.tile([P, P], F32)
            make_identity(nc, ident[:])
            y_sb = [ypool.tile([P, T], F16, name=f"y{h}", tag=f"y{h}") for h in range(HT)]

            # ---------------- Phase 0 + A ----------------
            with (
                tc.tile_pool(name="xT", bufs=1) as xpool,
                tc.tile_pool(name="wstage", bufs=2) as spool,
                tc.tile_pool(name="wf", bufs=2) as fpool,
                tc.tile_pool(name="psA", bufs=4, space="PSUM") as psA,
            ):
                xT = [xpool.tile([P, T], F16, name=f"xT{d}", tag=f"xT{d}") for d in range(DT)]

                # Phase 0: load x (T on partitions), PE-transpose to xT (fp16).
                for t in range(TT):
                    xs = spool.tile([P, D], F32, name="xs", tag="w1s")
                    nc.sync.dma_start(xs[:], x[t * P:(t + 1) * P, :])
                    for d in range(DT):
                        pt = psA.tile([P, P], F32, name="pt", tag="ps")
                        nc.tensor.transpose(pt[:], xs[:, d * P:(d + 1) * P], ident[:])
                        nc.vector.tensor_copy(xT[d][:, t * P:(t + 1) * P], pt[:])

                # Phase A: mm1/mm2 + SwiGLU per h-tile.
                for h in range(HT):
                    hs = slice(h * P, (h + 1) * P)
                    w1s = spool.tile([P, DT, P], F32, name="w1s", tag="w1s")
                    w2s = spool.tile([P, DT, P], F32, name="w2s", tag="w2s")
                    nc.sync.dma_start(
                        w1s[:], w1[:, hs].rearrange("(dt p) h -> p dt h", p=P))
                    nc.sync.dma_start(
                        w2s[:], w2[:, hs].rearrange("(dt p) h -> p dt h", p=P))
                    w1f = fpool.tile([P, DT, P], F16, name="w1f", tag="w1f")
                    w2f = fpool.tile([P, DT, P], F16, name="w2f", tag="w2f")
                    nc.vector.tensor_copy(w1f[:], w1s[:])
                    nc.vector.tensor_copy(w2f[:], w2s[:])

                    y1 = psA.tile([P, T], F32, name="y1", tag="ps")
                    y2 = psA.tile([P, T], F32, name="y2", tag="ps")
                    for half in range(2):
                        th = slice(half * 512, (half + 1) * 512)
                        for d in range(DT):
                            nc.tensor.matmul(
                                y1[:, th], lhsT=w1f[:, d, :], rhs=xT[d][:, th],
                                start=(d == 0), stop=(d == DT - 1))
                        for d in range(DT):
                            nc.tensor.matmul(
                                y2[:, th], lhsT=w2f[:, d, :], rhs=xT[d][:, th],
                                start=(d == 0), stop=(d == DT - 1))
                    s1 = fpool.tile([P, T], F32, name="s1", tag="s1")
                    nc.scalar.activation(s1[:], y1[:], mybir.ActivationFunctionType.Silu)
                    nc.vector.tensor_mul(y_sb[h][:], s1[:], y2[:])

            # ---------------- Phase B ----------------
            with (
                tc.tile_pool(name="w3stage", bufs=2) as s3pool,
                tc.tile_pool(name="w3f", bufs=2) as f3pool,
                tc.tile_pool(name="outs", bufs=4) as opool,
                tc.tile_pool(name="psB", bufs=4, space="PSUM") as psB,
            ):
                for db in range(NDB):
                    ds_ = slice(db * DB, (db + 1) * DB)
                    w3f = f3pool.tile([P, HT, DB], F16, name="w3f", tag="w3f")
                    for c in range(NCH):
                        w3s = s3pool.tile([P, CH, DB], F32, name="w3s", tag="w3s")
                        nc.sync.dma_start(
                            w3s[:],
                            w3[c * CH * P:(c + 1) * CH * P, ds_].rearrange(
                                "(ht p) d -> p ht d", p=P))
                        nc.vector.tensor_copy(w3f[:, c * CH:(c + 1) * CH, :], w3s[:])
                    for ts in range(TT):
                        po = psB.tile([P, DB], F32, name="po", tag="po")
                        for h in range(HT):
                            nc.tensor.matmul(
                                po[:], lhsT=y_sb[h][:, ts * P:(ts + 1) * P],
                                rhs=w3f[:, h, :],
                                start=(h == 0), stop=(h == HT - 1))
                        ob = opool.tile([P, DB], F32, name="ob", tag="ob")
                        nc.scalar.activation(
                            ob[:], po[:], mybir.ActivationFunctionType.Copy)
                        nc.sync.dma_start(out[ts * P:(ts + 1) * P, ds_], ob[:])

    nc.compile()
    return nc


def kernel(x, W_fc1, W_fc2, W_fc3, trace=False, trace_cores=None):
    if "nc" not in _cache:
        _cache["nc"] = _build()
    nc = _cache["nc"]

    in_maps = [
        {
            "x": np.ascontiguousarray(x[e]),
            "w1": np.ascontiguousarray(W_fc1[e]),
            "w2": np.ascontiguousarray(W_fc2[e]),
            "w3": np.ascontiguousarray(W_fc3[e]),
        }
        for e in range(E)
    ]
    res = run_bass_kernel_spmd(
        nc, in_maps, core_ids=list(range(E)),
        trace=trace, trace_cores=trace_cores,
    )
    out = np.stack([res.results[e]["out"] for e in range(E)])
    if trace:
        kernel.last_result = res
    return out
